# revision 26
# baseline (speedup 1.0000x reference)
"""Bass/Trainium2 kernel for attention-energy softmax (fp16-stream variant):
  proj = enc @ W.T + b        [S,B,D]
  energies[b,s] = hidden[b] . proj[s,b]
  out = softmax(energies, axis=s)[:, None, :]

Algebraic fusion: energies[b,s] = (hidden[b] @ W) . enc[s,b] + hidden[b].b
The bias term is constant per b and cancels in softmax, so it is dropped.

The inputs are cast to fp16 during host-side sharding (measured output
rel-L2 ~2.0e-3, 10x inside the 2e-2 gate): this halves the HBM stream
(the steady-state limiter is the 716 GB/s HBM stack shared by each core
pair) and unlocks the DVE's 2x packed-fp16 tensor_tensor mode. The PE
runs fp16 natively (1-pass, vs 4-pass fp32), accumulating v = hidden@W
in fp32 PSUM; energy accumulation is fp32 throughout (stt accumulator /
ACT accum_out), so only input rounding and the product tiles are 16-bit.

Per-tile steady state (~7us each, DVE/ACT/DMA balanced):
  - DMA: 2MB fp16 tile (16KB/partition contiguous rows)
  - DVE: NSTT b's fused stt multiply+accum (1x rate, but one op) and
    the rest as tensor_tensor fp16 multiplies (2x mode, ~0.7us)
  - ACT: the remaining b's reduced by Copy-with-accum_out over the
    rotating prod tiles
Softmax: two overlapped b-groups, cross-partition max/sum via PE
transpose / ones-matmul; the last tile runs all-stt so the ACT queue is
clear for the exps, and the normalization is folded into the PSUM-drain
copies, split across DVE and ACT.
"""

import numpy as np

import concourse.bass as bass
import concourse.mybir as mybir
from concourse import bacc
from concourse.masks import make_identity
from concourse.bass_utils import run_bass_kernel_spmd
from concourse.tile import TileContext

S, B, D = 2048, 64, 1024
NCORES = 8
BL = B // NCORES  # 8 local batches per core
P = 128
T = S // P  # 16 seq tiles
EC = D // P  # 8 contraction chunks
F32 = mybir.dt.float32
F16 = mybir.dt.float16
BF16 = mybir.dt.bfloat16
NSTT = 3  # b's per tile handled by fused stt on DVE (rest: tt + ACT accum)

TRACE = False  # test harness sets True to profile

_CACHE = {}


def _stats_a(nc, small, pstat, ident, ones8, e_all, g0, gw):
    """Cross-partition max chain for b in [g0, g0+gw): returns the
    negated group maxima broadcast to all partitions (DVE/PE only)."""
    m8g = small.tile([P, gw], F32, tag=f"m8{g0}")
    nc.vector.tensor_reduce(
        out=m8g,
        in_=e_all[:, g0 : g0 + gw, :],
        axis=mybir.AxisListType.X,
        op=mybir.AluOpType.max,
    )
    trm = pstat.tile([gw, P], F32, tag="stat")
    nc.tensor.transpose(trm, m8g, ident)
    mt = small.tile([gw, P], F32, tag=f"mt{g0}")
    nc.vector.tensor_copy(out=mt, in_=trm)
    gmax = small.tile([gw, 1], F32, tag=f"gmax{g0}")
    nc.vector.tensor_reduce(
        out=gmax, in_=mt, axis=mybir.AxisListType.X, op=mybir.AluOpType.max
    )
    gneg = small.tile([gw, 1], F32, tag=f"gneg{g0}")
    nc.vector.tensor_scalar_mul(gneg, gmax, -1.0)
    diag = small.tile([gw, gw], F32, tag=f"diag{g0}")
    nc.vector.tensor_scalar_mul(diag, ident[0:gw, 0:gw], gneg)
    ngps = pstat.tile([P, gw], F32, tag="stat")
    nc.tensor.matmul(ngps, ones8[0:gw, :], diag, start=True, stop=True)
    negg = small.tile([P, gw], F32, tag=f"negg{g0}")
    nc.vector.tensor_copy(out=negg, in_=ngps)
    return negg


def _stats_exp(nc, small, e_all, negg, g0, gw):
    """exp(e - max) in place with accumulated sums (ACT only)."""
    s8g = small.tile([P, gw], F32, tag=f"s8{g0}")
    for j in range(gw):
        b = g0 + j
        nc.scalar.activation(
            out=e_all[:, b, :],
            in_=e_all[:, b, :],
            func=mybir.ActivationFunctionType.Exp,
            bias=negg[:, j : j + 1],
            accum_out=s8g[:, j : j + 1],
        )
    return s8g


def _recip_bcast(nc, small, pstat, ones_col, ones_row, s8g, gw):
    """1/sum per b, broadcast to all partitions via K=1 ones-matmul."""
    smps = pstat.tile([1, gw], F32, tag="stat")
    nc.tensor.matmul(smps, ones_col, s8g, start=True, stop=True)
    srow = small.tile([1, gw], F32, tag="srow")
    nc.vector.tensor_copy(out=srow, in_=smps)
    rrow = small.tile([1, gw], F32, tag="rrow")
    nc.vector.reciprocal(rrow, srow)
    rps = pstat.tile([P, gw], F32, tag="stat")
    nc.tensor.matmul(rps, ones_row, rrow, start=True, stop=True)
    recipg = small.tile([P, gw], F32, tag="recip")
    nc.vector.tensor_copy(out=recipg, in_=rps)
    return recipg


def build_kernel() -> bass.Bass:
    nc = bacc.Bacc(None, target_bir_lowering=False)
    enc = nc.dram_tensor("enc", [S, BL, D], F16, kind="ExternalInput")
    wx = nc.dram_tensor("wx", [P, EC * (D + BL)], F16, kind="ExternalInput")
    out = nc.dram_tensor("out", [BL, S], F32, kind="ExternalOutput")
    DB = D + BL

    with TileContext(nc) as tc:
        with (
            tc.tile_pool(name="consts", bufs=1) as consts,
            tc.tile_pool(name="work", bufs=6) as work,
            tc.tile_pool(name="small", bufs=2) as small,
            tc.tile_pool(name="mm", bufs=2, space="PSUM") as mmp,
            tc.tile_pool(name="ptr", bufs=2, space="PSUM") as ptr,
            tc.tile_pool(name="pstat", bufs=2, space="PSUM") as pstat,
        ):
            # ---- load [W|hT] (fp32), host-packed so each partition row is
            # one contiguous 33KB read; first DMAs in the queue ----
            wx_r = wx[:, :].rearrange("p (c f) -> p c f", f=DB)
            wt = work.tile([P, EC, DB], F16, tag="wx", bufs=1)
            nc.sync.dma_start(out=wt[:, 0 : EC // 2, :], in_=wx_r[:, 0 : EC // 2, :])
            nc.sync.dma_start(out=wt[:, EC // 2 : EC, :], in_=wx_r[:, EC // 2 : EC, :])
            wx_sb = [wt[:, c : c + 1, :] for c in range(EC)]

            # enc DMAs: tile 0 split into b-quarters for an early start
            GW = BL // 2  # softmax / pipeline group width
            enc_sb = []
            t0 = work.tile([P, BL, D], F16, tag="enc_t")
            for q in range(4):
                nc.sync.dma_start(
                    out=t0[:, 2 * q : 2 * q + 2, :],
                    in_=enc[0:P, 2 * q : 2 * q + 2, :],
                )
            enc_sb.append(t0)

            ident = consts.tile([P, P], F32)
            make_identity(nc, ident)
            # Warm the PE p-state while the weight DMAs are in flight.
            warm_ps = mmp.tile([P, 512], F32, tag="bc")
            for _ in range(8):
                nc.tensor.matmul(
                    warm_ps[:, 0:P], ident, ident, start=True, stop=True
                )

            # selector tiles: sel[k, b, m] = 1 if k == b else 0
            ones8 = consts.tile([BL, P], F32)
            nc.vector.memset(ones8, 1.0)
            ones8b = consts.tile([BL, P], F16)
            nc.vector.memset(ones8b, 1.0)
            sel = consts.tile([BL, BL, P], F16)
            for b in range(BL):
                nc.vector.tensor_scalar_mul(
                    sel[:, b, :], ones8b, ident[0:BL, b : b + 1]
                )

            # v = hidden_local @ W -> [BL, D] (fp32 on the PE)
            v_sb = consts.tile([BL, D], F16)
            v_ps0 = mmp.tile([BL, 512], F32, tag="mm")
            v_ps1 = mmp.tile([BL, 512], F32, tag="mm")
            for c in range(EC):
                for h, v_ps in ((0, v_ps0), (1, v_ps1)):
                    nc.tensor.matmul(
                        v_ps,
                        wx_sb[c][:, 0, D : D + BL],
                        wx_sb[c][:, 0, h * 512 : (h + 1) * 512],
                        start=(c == 0),
                        stop=(c == EC - 1),
                    )
            nc.scalar.copy(out=v_sb[:, 0:512], in_=v_ps0)
            nc.scalar.copy(out=v_sb[:, 512:1024], in_=v_ps1)

            # ---- broadcast v to all partitions, cast fp16 on the drain:
            # vb16[p, b, d] = fp16(v[b, d]) ----
            vb16 = consts.tile([P, BL, D], F16)

            def vb_bcast(b, eng, pool, tag):
                for h in range(2):
                    bc_ps = pool.tile([P, 512], F32, tag=tag)
                    nc.tensor.matmul(
                        bc_ps,
                        sel[:, b, :],
                        v_sb[:, h * 512 : (h + 1) * 512],
                        start=True,
                        stop=True,
                    )
                    eng(out=vb16[:, b, h * 512 : (h + 1) * 512], in_=bc_ps)

            for b in range(4):
                vb_bcast(b, nc.scalar.copy, mmp, "bc")

            # ---- energies: e_all[p, b, t] = sum_d enc[t*128+p, b, d]*v[b, d] ----
            e_all = consts.tile([P, BL, T], F32)
            # unused elementwise outputs go to PSUM (stt) / a dummy fp16
            # broadcast (ACT) so they don't eat SBUF write bandwidth
            dummy = mmp.tile([P, 1], F32, tag="mm")
            dummy16 = consts.tile([P, 1], F16)
            ones_col = consts.tile([P, 1], F32)
            nc.vector.memset(ones_col, 1.0)
            ones_row = consts.tile([1, P], F32)
            nc.vector.memset(ones_row, 1.0)
            out_r = out[:, :].rearrange("b (t p) -> t b p", p=P)
            out_t = consts.tile([T, BL, P], F32)

            def stt(src, j, b, t):
                # fused multiply + free-dim sum in one DVE pass (1x rate)
                nc.vector.scalar_tensor_tensor(
                    out=dummy.broadcast_to((P, D)),
                    in0=src[:, j, :],
                    scalar=1.0,
                    in1=vb16[:, b, :],
                    op0=mybir.AluOpType.mult,
                    op1=mybir.AluOpType.mult,
                    accum_out=e_all[:, b, t : t + 1],
                )

            def tt_acc(src, j, b, t):
                # 2x-mode fp16 multiply on DVE, reduction on ACT
                prod = work.tile([P, D], F16, tag="prod", bufs=3)
                nc.vector.tensor_tensor(
                    out=prod,
                    in0=src[:, j, :],
                    in1=vb16[:, b, :],
                    op=mybir.AluOpType.mult,
                )
                nc.scalar.activation(
                    out=dummy16.broadcast_to((P, D)),
                    in_=prod,
                    func=mybir.ActivationFunctionType.Copy,
                    accum_out=e_all[:, b, t : t + 1],
                )

            def consume_tile(tc_):
                # first NSTT b's via fused stt (DVE-only), the rest via
                # tt + ACT reduce; on tile 0 the DVE also drains vb b4-7
                # between its own ops.
                for b in range(BL):
                    if tc_ == 0 and b < 4:
                        vb_bcast(b + 4, nc.vector.tensor_copy, pstat, "stat")
                    if b < NSTT:
                        stt(enc_sb[tc_], b, b, tc_)
                    else:
                        tt_acc(enc_sb[tc_], b, b, tc_)

            # issue DMAs for tiles 1..5 up front (work pool holds 6), then
            # rotate: DMA for tile t+5 is issued as tile t is consumed.
            for t in range(1, T - 1):
                enc_t = work.tile([P, BL, D], F16, tag="enc_t")
                nc.sync.dma_start(out=enc_t, in_=enc[t * P : (t + 1) * P, :, :])
                enc_sb.append(enc_t)
                if t >= 5:
                    consume_tile(t - 5)
            for tc_ in range(T - 6, T - 1):
                consume_tile(tc_)
                if tc_ == T - 2:
                    # re-warm the PE p-state for the softmax transposes
                    wp = mmp.tile([P, 512], F32, tag="bc")
                    nc.tensor.matmul(
                        wp[:, 0:8], ident, e_all[:, :, 14],
                        start=True, stop=True,
                    )
                    for w in range(8):
                        wp = mmp.tile([P, 512], F32, tag="bc")
                        nc.tensor.matmul(
                            wp[:, 0:P], ident, ident, start=True, stop=True
                        )

            # last seq tile split by b-halves so group-0 softmax overlaps
            # the remaining multiply stream
            t = T - 1
            tl = work.tile([P, BL, D], F16, tag="enc_t")
            for gh in range(2):
                nc.sync.dma_start(
                    out=tl[:, gh * GW : (gh + 1) * GW, :],
                    in_=enc[t * P : (t + 1) * P, gh * GW : (gh + 1) * GW, :],
                )
            # The last tile runs all-stt (DVE only) so the ACT queue is
            # clear for the softmax exps; stats phase A (max chain) for
            # group 0 is issued between the halves, exps after both, so
            # neither group's chain blocks the other's engine queue.
            for j in range(GW):
                stt(tl, j, j, t)
            negg0 = _stats_a(nc, small, pstat, ident, ones8, e_all, 0, GW)
            for j in range(GW):
                stt(tl, GW + j, GW + j, t)
            s8_0 = _stats_exp(nc, small, e_all, negg0, 0, GW)
            negg1 = _stats_a(nc, small, pstat, ident, ones8, e_all, GW, GW)
            s8_1 = _stats_exp(nc, small, e_all, negg1, GW, GW)
            # Transposes depend only on the exp'd energies, so the PE works
            # in parallel with the reciprocal chain; the normalization is
            # folded into the drain copies (ACT via scale operand, DVE via
            # per-partition tensor_scalar), split across both engines.
            for g0, s8g in ((0, s8_0), (GW, s8_1)):
                recipg = _recip_bcast(
                    nc, small, pstat, ones_col, ones_row, s8g, GW
                )
                for j in range(GW):
                    b = g0 + j
                    tr = ptr.tile([T, P], F32, tag="tr")
                    nc.tensor.transpose(tr, e_all[:, b, :], ident)
                    if j % 2 == 0:
                        nc.vector.tensor_scalar_mul(
                            out_t[:, b, :], tr, recipg[0:T, j : j + 1]
                        )
                    else:
                        nc.scalar.activation(
                            out=out_t[:, b, :],
                            in_=tr,
                            func=mybir.ActivationFunctionType.Copy,
                            scale=recipg[0:T, j : j + 1],
                        )
                nc.sync.dma_start(
                    out=out_r[:, g0 : g0 + GW, :], in_=out_t[:, g0 : g0 + GW, :]
                )

    nc.compile()
    return nc


def kernel(hidden, encoder_outputs, W_attn, b_attn):
    hidden = np.asarray(hidden, dtype=np.float32)
    W_attn = np.asarray(W_attn, dtype=np.float32)
    enc16 = np.asarray(encoder_outputs, dtype=np.float16)

    in_maps = []
    for c in range(NCORES):
        bs = slice(c * BL, (c + 1) * BL)
        wx_full = np.concatenate([W_attn, hidden[0, bs, :].T], axis=1)
        wxp = np.ascontiguousarray(
            wx_full.reshape(EC, P, D + BL).transpose(1, 0, 2).reshape(P, -1)
        ).astype(np.float16)
        in_maps.append(
            {
                "enc": np.ascontiguousarray(enc16[:, bs, :]),
                "wx": wxp,
            }
        )

    if "nc" not in _CACHE:
        _CACHE["nc"] = build_kernel()
    nc = _CACHE["nc"]

    res = run_bass_kernel_spmd(nc, in_maps, core_ids=list(range(NCORES)), trace=TRACE)
    if TRACE:
        _CACHE["last_result"] = res
    out = np.concatenate([r["out"] for r in res.results], axis=0)  # [B, S]
    return out[:, None, :]
